# revision 7
# baseline (speedup 1.0000x reference)
"""Grouped-Query Attention kernel for 8 Trainium2 NeuronCores.

Sharding: 8 cores = (batch b in {0,1}) x (KV-head-pair group g in {0..3}).
Each core handles batch b, KV heads {2g, 2g+1}, Q heads {8g..8g+7}:
  - Q/K/V projections (column-sharded weights)
  - scores S = Qp K^T / 8 in BOTH orientations:
      S   [q,k] for the attn_weights output (softmax'd, DMA'd out)
      S^T [k,q] so exp(S^T) can feed P@V directly as the stationary-side
      contraction without any on-chip transpose of the 1 GiB P matrix
  - attn output, then row-sharded output projection -> partial output
Host: pre-transposes activations (contraction dim must sit on SBUF
partitions), permutes Wq cols / Wo rows so heads (t, t+4) pair up in
128-partition tiles (enables row/col-tiled K=64 matmul packing), sums the
4 output partials per batch and adds bo (the out-proj all-reduce).

Heads pair as (t, t+4) because head t uses KV head 0 (kp rows 0:64) and
head t+4 uses KV head 1 (kp rows 64:128) - both operands of the packed
matmuls then sit at the base_partition their tile_position requires.

key_padding_mask is all-False for this problem spec (fill=zeros), so the
mask is a no-op and is ignored on device.
"""

import os
import numpy as np

import concourse.bass as bass
import concourse.mybir as mybir
import concourse.tile as tile
from concourse import bacc
from concourse.bass_utils import run_bass_kernel_spmd

F32 = mybir.dt.float32
# float32r = fp32 bits, reduced-precision PE path (1 cyc/row vs 4 for fp32).
# BIR requires f32r matmul inputs to be *produced* as f32r, so every tensor
# that feeds a matmul is declared f32r end-to-end (same bytes as fp32).
MM_DT = mybir.dt.float32r if os.environ.get("GQA_MM_FP32", "0") != "1" else F32
MR = MM_DT

E = 2048
D = 64          # head dim
B, LQ, LKV = 2, 2048, 2048
DQ = 512        # per-core Q dims   (8 heads)
DKV = 128       # per-core KV dims  (2 KV heads)
SCALE = 0.125   # 1/sqrt(64)

BF16 = mybir.dt.bfloat16
Exp = mybir.ActivationFunctionType.Exp
ts = bass.ts


def _mm(ap):
    """View an fp32 AP as the matmul input dtype (same bytes)."""
    return ap


def build_nc():
    nc = bacc.Bacc(None, target_bir_lowering=False)

    qT = nc.dram_tensor("qT", [E, LQ], MR, kind="ExternalInput")
    kT = nc.dram_tensor("kT", [E, LKV], MR, kind="ExternalInput")
    vT = nc.dram_tensor("vT", [E, LKV], MR, kind="ExternalInput")
    wq = nc.dram_tensor("wq", [E, DQ], MR, kind="ExternalInput")
    wk = nc.dram_tensor("wk", [E, DKV], MR, kind="ExternalInput")
    wv = nc.dram_tensor("wv", [E, DKV], MR, kind="ExternalInput")
    wo = nc.dram_tensor("wo", [DQ, E], MR, kind="ExternalInput")
    attn = nc.dram_tensor("attn", [8, LQ, LKV], F32, kind="ExternalOutput")
    outp = nc.dram_tensor("outp", [LQ, E], F32, kind="ExternalOutput")

    with tile.TileContext(nc) as tc:
        with (
            tc.tile_pool(name="persist", bufs=1) as persist,
            tc.tile_pool(name="acts", bufs=4) as acts,
            tc.tile_pool(name="wstream", bufs=3) as wstream,
            tc.tile_pool(name="expp", bufs=4) as expp,
            tc.tile_pool(name="pout", bufs=3) as pout,
            tc.tile_pool(name="small", bufs=4) as small,
            tc.tile_pool(name="rbp", bufs=2) as rbp,
            tc.tile_pool(name="psum", bufs=4, space="PSUM") as psum,
            tc.tile_pool(name="dram", bufs=2, space="DRAM") as dram,
        ):
            # ---- persistent SBUF state
            qp = [persist.tile([128, LQ], MR, name=f"qp{t}", tag=f"qp{t}") for t in range(4)]
            kp = persist.tile([128, LKV], MR, name="kp", tag="kp")
            vp = persist.tile([128, LKV], BF16, name="vp", tag="vp")   # [tok%128, kc*128+dv]
            attnT = [persist.tile([128, LQ], MR, name=f"at{t}", tag=f"at{t}") for t in range(4)]
            wk_sb = persist.tile([128, 16, DKV], MR, name="wk", tag="wk")
            wv_sb = persist.tile([128, 16, DKV], MR, name="wv", tag="wv")
            recip = [persist.tile([128, 16], F32, name=f"rc{j}", tag=f"rc{j}") for j in range(8)]
            ident = persist.tile([128, 128], F32, name="ident", tag="ident")

            from concourse.masks import make_identity
            make_identity(nc, ident[:, :])

            nc.sync.dma_start(
                out=wk_sb[:, :, :],
                in_=wk[:, :].rearrange("(ec p) c -> p ec c", p=128),
            )
            nc.sync.dma_start(
                out=wv_sb[:, :, :],
                in_=wv[:, :].rearrange("(ec p) c -> p ec c", p=128),
            )

            # ---- phase 1a: Q projection  QpT[t] = (query @ Wq)^T pair-tile t
            for n in range(4):                      # token cols, 512 each
                psq = [psum.tile([128, 512], F32, name="score", tag="score") for _ in range(4)]
                for e in range(16):                 # contraction chunks
                    qts = acts.tile([128, 512], MR, name="qts", tag="qts")
                    nc.sync.dma_start(qts[:, :], qT[ts(e, 128), ts(n, 512)])
                    wqt = wstream.tile([128, 512], MR, name="wqs", tag="wqs")
                    nc.sync.dma_start(wqt[:, :], wq[ts(e, 128), :])
                    for t in range(4):
                        nc.tensor.matmul(
                            psq[t][:, :],
                            _mm(wqt[:, ts(t, 128)]),
                            _mm(qts[:, :]),
                            start=(e == 0),
                            stop=(e == 15),
                        )
                for t in range(4):
                    nc.vector.tensor_copy(qp[t][:, ts(n, 512)], psq[t][:, :])

            # ---- phase 1b: K projection  kp = (key @ Wk)^T
            for n in range(4):
                psk = psum.tile([128, 512], F32, name="score", tag="score")
                for e in range(16):
                    kts = acts.tile([128, 512], MR, name="kts", tag="kts")
                    nc.sync.dma_start(kts[:, :], kT[ts(e, 128), ts(n, 512)])
                    nc.tensor.matmul(
                        psk[:, :],
                        _mm(wk_sb[:, e, :]),
                        _mm(kts[:, :]),
                        start=(e == 0),
                        stop=(e == 15),
                    )
                nc.vector.tensor_copy(kp[:, ts(n, 512)], psk[:, :])

            # ---- phase 1c: V projection  vp chunk mt = value[mt*128:...] @ Wv
            for vg in range(4):                     # token groups of 512
                psv = [psum.tile([128, DKV], F32, name="score", tag="score") for _ in range(4)]
                for e in range(16):
                    vts = acts.tile([128, 512], MR, name="vts", tag="vts")
                    nc.sync.dma_start(vts[:, :], vT[ts(e, 128), ts(vg, 512)])
                    for mi in range(4):
                        nc.tensor.matmul(
                            psv[mi][:, :],
                            _mm(vts[:, ts(mi, 128)]),
                            _mm(wv_sb[:, e, :]),
                            start=(e == 0),
                            stop=(e == 15),
                        )
                for mi in range(4):
                    mt = vg * 4 + mi
                    nc.vector.tensor_copy(vp[:, ts(mt, 128)], psv[mi][:, :])

            # ---- phases 2-4 per head pair t: heads (jA, jB) = (t, t+4)
            for t in range(4):
                jA, jB = t, t + 4

                # phase 2: S^T chunks -> exp -> P@V (unnormalized attnT)
                for qc in range(4):                 # q cols, 512 each
                    avA = psum.tile([128, 512], F32, name="av", tag="av")
                    avB = psum.tile([128, 512], F32, name="av", tag="av")
                    for kc in range(16):            # k rows, 128 each
                        stA = psum.tile([128, 512], F32, name="score", tag="score")
                        stB = psum.tile([128, 512], F32, name="score", tag="score")
                        nc.tensor.matmul(
                            stA[:, :],
                            _mm(kp[0:64, ts(kc, 128)]),
                            _mm(qp[t][0:64, ts(qc, 512)]),
                            tile_position=(0, 0),
                        )
                        nc.tensor.matmul(
                            stB[:, :],
                            _mm(kp[64:128, ts(kc, 128)]),
                            _mm(qp[t][64:128, ts(qc, 512)]),
                            tile_position=(64, 0),
                        )
                        eA = expp.tile([128, 512], BF16, name="expT", tag="expT")
                        eB = expp.tile([128, 512], BF16, name="expT", tag="expT")
                        nc.scalar.activation(eA[:, :], stA[:, :], Exp, scale=SCALE)
                        nc.scalar.activation(eB[:, :], stB[:, :], Exp, scale=SCALE)
                        nc.tensor.matmul(
                            avA[0:64, :],
                            _mm(vp[:, kc * 128: kc * 128 + 64]),
                            _mm(eA[:, :]),
                            start=(kc == 0),
                            stop=(kc == 15),
                            tile_position=(0, 0),
                        )
                        nc.tensor.matmul(
                            avB[64:128, :],
                            _mm(vp[:, kc * 128 + 64: kc * 128 + 128]),
                            _mm(eB[:, :]),
                            start=(kc == 0),
                            stop=(kc == 15),
                            tile_position=(0, 64),
                        )
                    nc.vector.tensor_copy(attnT[t][0:64, ts(qc, 512)], avA[0:64, :])
                    nc.vector.tensor_copy(attnT[t][64:128, ts(qc, 512)], avB[64:128, :])

                # phase 3: S -> exp (+rowsum) -> normalize -> attn out
                for qc in range(16):                # q rows, 128 each
                    for j, rows in ((jA, slice(0, 64)), (jB, slice(64, 128))):
                        ptile = pout.tile([128, LKV], F32, name="p", tag="p")
                        acc = small.tile([128, 4], F32, name="acc", tag="acc")
                        for kc in range(4):         # k cols, 512 each
                            ps = psum.tile([128, 512], F32, name="score", tag="score")
                            nc.tensor.matmul(
                                ps[:, :],
                                _mm(qp[t][rows, ts(qc, 128)]),
                                _mm(kp[rows, ts(kc, 512)]),
                            )
                            nc.scalar.activation(
                                ptile[:, ts(kc, 512)], ps[:, :], Exp,
                                scale=SCALE, accum_out=acc[:, kc:kc + 1],
                            )
                        rs = small.tile([128, 1], F32, name="rs", tag="rs")
                        nc.vector.tensor_reduce(
                            rs[:, :], acc[:, :],
                            axis=mybir.AxisListType.X, op=mybir.AluOpType.add,
                        )
                        nc.vector.reciprocal(recip[j][:, qc:qc + 1], rs[:, :])
                        nc.vector.tensor_scalar_mul(
                            ptile[:, :], ptile[:, :], recip[j][:, qc:qc + 1]
                        )
                        nc.sync.dma_start(attn[j, ts(qc, 128), :], ptile[:, :])

                # phase 4: normalize attnT by 1/rowsum (q is the free dim, so
                # transpose the recip vectors and broadcast across partitions)
                rb = rbp.tile([128, LQ], F32, name="rb", tag="rb")
                for j, rows in ((jA, slice(0, 64)), (jB, slice(64, 128))):
                    pst = psum.tile([16, 128], F32, name="score", tag="score")
                    nc.tensor.transpose(pst[:, :], recip[j][:, :], ident[:, :])
                    r16 = small.tile([16, 128], F32, name="r16", tag="r16")
                    nc.vector.tensor_copy(r16[:, :], pst[:, :])
                    bounce = dram.tile([16, 128], F32, name="bounce", tag="bounce")
                    nc.sync.dma_start(bounce[:, :], r16[:, :])
                    bap = bounce[:, :]
                    bcast = bass.AP(
                        tensor=bap.tensor, offset=bap.offset,
                        ap=[[0, 64], [1, LQ]],
                    )
                    nc.gpsimd.dma_start(out=rb[rows, :], in_=bcast)
                nc.vector.tensor_mul(attnT[t][:, :], attnT[t][:, :], rb[:, :])

            # ---- phase 5: output projection (row-parallel partial)
            for ec in range(4):                     # out cols, 512 each
                wot = [wstream.tile([128, 512], MR, name="wo", tag="wo", bufs=8)
                       for _ in range(4)]
                for dc in range(4):
                    nc.sync.dma_start(wot[dc][:, :], wo[ts(dc, 128), ts(ec, 512)])
                for qs in range(16):                # q rows, 128 each
                    pso = psum.tile([128, 512], F32, name="av", tag="av")
                    for dc in range(4):
                        nc.tensor.matmul(
                            pso[:, :],
                            _mm(attnT[dc][:, ts(qs, 128)]),
                            _mm(wot[dc][:, :]),
                            start=(dc == 0),
                            stop=(dc == 3),
                        )
                    osb = pout.tile([128, 512], F32, name="osb", tag="osb")
                    nc.vector.tensor_copy(osb[:, :], pso[:, :])
                    nc.sync.dma_start(outp[ts(qs, 128), ts(ec, 512)], osb[:, :])

    nc.compile()
    return nc


_NC_CACHE = None
LAST_RESULTS = None
LAST_IN_MAPS = None


def _get_nc():
    global _NC_CACHE
    if _NC_CACHE is None:
        _NC_CACHE = build_nc()
    return _NC_CACHE


def kernel(query, key, value, key_padding_mask, Wq, Wk, Wv, Wo, bo):
    global LAST_RESULTS, LAST_IN_MAPS
    query = np.asarray(query, np.float32)
    key = np.asarray(key, np.float32)
    value = np.asarray(value, np.float32)
    Wq = np.asarray(Wq, np.float32)
    Wk = np.asarray(Wk, np.float32)
    Wv = np.asarray(Wv, np.float32)
    Wo = np.asarray(Wo, np.float32)
    bo = np.asarray(bo, np.float32)

    qTb = [np.ascontiguousarray(query[b].T) for b in range(B)]
    kTb = [np.ascontiguousarray(key[b].T) for b in range(B)]
    vTb = [np.ascontiguousarray(value[b].T) for b in range(B)]

    in_maps = []
    perms = []
    for core in range(8):
        b, g = divmod(core, 4)
        # head pair tile t holds dims of global heads (8g+t, 8g+t+4)
        perm = np.concatenate([
            np.arange(64 * (8 * g + t + o), 64 * (8 * g + t + o) + 64)
            for t in range(4) for o in (0, 4)
        ])
        perms.append(perm)
        in_maps.append({
            "qT": qTb[b], "kT": kTb[b], "vT": vTb[b],
            "wq": np.ascontiguousarray(Wq[:, perm]),
            "wk": np.ascontiguousarray(Wk[:, 128 * g: 128 * g + 128]),
            "wv": np.ascontiguousarray(Wv[:, 128 * g: 128 * g + 128]),
            "wo": np.ascontiguousarray(Wo[perm, :]),
        })

    LAST_IN_MAPS = in_maps
    nc = _get_nc()
    LAST_RESULTS = run_bass_kernel_spmd(nc, in_maps, core_ids=list(range(8)))
    res = LAST_RESULTS.results

    attn_w = np.empty((B, 32, LQ, LKV), np.float32)
    output = np.zeros((B, LQ, E), np.float32)
    for core in range(8):
        b, g = divmod(core, 4)
        r = res[core]
        for j in range(8):
            attn_w[b, 8 * g + j] = r["attn"][j]
        output[b] += r["outp"]
    output += bo
    return output, attn_w
